# revision 10
# baseline (speedup 1.0000x reference)
"""Causal multi-head attention on 8 TRN2 NeuronCores.

Problem: B=4, S=2048, D=1024, H=16, HD=64, causal MHA with out-proj + bias.

Sharding: core c handles (batch b = c//2, head-half hh = c%2), i.e. 8 heads of
one batch element. Per core:
  Q^T/K^T = (Wq/Wk half)^T X_b^T   -> [64*2, S] per head pair (head on partition)
  V       = X_b @ Wv half          -> [S, 8*65] (65 = 64 + fused-ones column)
  S^T_j   = K_j Q^T (scores transposed: keys on partition) per 128-key block
  P^T     = exp(S^T/8) (ScalarE), causal handled by block skipping + one
            [128,128] additive mask on diagonal blocks
  ctx'^T  = V'^T P^T accumulated over key blocks; row 64 = softmax denominators
            (ones-column trick)
  ctxT    = ctx'^T * (1/denom) broadcast  (GPSIMD partition_broadcast + DVE)
  out     = ctxT^T @ Wo half  (partial; host sums the two half partials + bias)

All matmuls in bf16 (fp32 accumulate in PSUM); scores/softmax in fp32.
"""

import numpy as np
import ml_dtypes
from contextlib import ExitStack

import concourse.bass as bass
import concourse.bacc as bacc
import concourse.mybir as mybir
import concourse.tile as tile
from concourse import bass_utils

F32 = mybir.dt.float32
BF16 = mybir.dt.bfloat16

B, S, D = 4, 2048, 1024
H, HD = 16, 64
DH = 512          # columns of the head-half handled by one core (8 heads * 64)
NCORES = 8
CH = 512          # q chunk width
NCH = S // CH     # 4
NKB = S // 128    # 16 key/query 128-blocks
KT = D // 128     # 8 contraction tiles for the projections
NEG = -30000.0    # additive mask value (exp(0.125*NEG) == 0 in fp32)

_CACHED = None


def build_module():
    nc = bacc.Bacc("TRN2", target_bir_lowering=False, debug=False)

    xT = nc.dram_tensor("xT", [D, S], BF16, kind="ExternalInput")
    wq = nc.dram_tensor("wq", [D, DH], BF16, kind="ExternalInput")
    wk = nc.dram_tensor("wk", [D, DH], BF16, kind="ExternalInput")
    wv = nc.dram_tensor("wv", [D, DH], BF16, kind="ExternalInput")
    wo = nc.dram_tensor("wo", [DH, D], BF16, kind="ExternalInput")
    maskt = nc.dram_tensor("maskt", [128, 128], F32, kind="ExternalInput")
    out = nc.dram_tensor("out", [S, D], F32, kind="ExternalOutput")
    # DRAM bounce rows for the reciprocal partition-broadcast (DMA cannot
    # broadcast an SBUF source, but a DRAM source row replicates fine)
    rscratch = nc.dram_tensor("rscratch", [4 * NCH * 2, CH], F32, kind="Internal")

    with tile.TileContext(nc) as tc, ExitStack() as ctx:
        const = ctx.enter_context(tc.tile_pool(name="const", bufs=1))
        xTp = ctx.enter_context(tc.tile_pool(name="xTp", bufs=1))
        wp = ctx.enter_context(tc.tile_pool(name="wp", bufs=1))
        qkp = ctx.enter_context(tc.tile_pool(name="qkp", bufs=1))
        vp = ctx.enter_context(tc.tile_pool(name="vp", bufs=1))
        ctp = ctx.enter_context(tc.tile_pool(name="ctp", bufs=1))
        pTp = ctx.enter_context(tc.tile_pool(name="pTp", bufs=6))
        rp = ctx.enter_context(tc.tile_pool(name="rp", bufs=3))
        bp = ctx.enter_context(tc.tile_pool(name="bp", bufs=3))
        osb = ctx.enter_context(tc.tile_pool(name="osb", bufs=3))
        ps_sc = ctx.enter_context(tc.tile_pool(name="ps_sc", bufs=4, space="PSUM"))
        ps_ctx = ctx.enter_context(tc.tile_pool(name="ps_ctx", bufs=2, space="PSUM"))
        ps_mm = ctx.enter_context(tc.tile_pool(name="ps_mm", bufs=2, space="PSUM"))

        mask = const.tile([128, 128], F32, name="mask", tag="mask")
        nc.sync.dma_start(mask[:], maskt[:])

        # --- input loads -------------------------------------------------
        xts = []
        for k in range(KT):
            t = xTp.tile([128, S], BF16, name=f"xT{k}", tag=f"xT{k}")
            xts.append(t)
        # chunked loads so early compute can start before the whole X^T lands
        for k in range(KT):
            for c in range(NCH):
                nc.sync.dma_start(xts[k][:, CH * c:CH * (c + 1)],
                                  xT[128 * k:128 * (k + 1), CH * c:CH * (c + 1)])

        def load_w(name, dram, cols):
            ts = []
            for k in range(dram.shape[0] // 128):
                t = wp.tile([128, cols], BF16, name=f"{name}{k}", tag=f"{name}{k}")
                nc.sync.dma_start(t[:], dram[128 * k:128 * (k + 1), :])
                ts.append(t)
            return ts

        wq_t = load_w("wq", wq, DH)
        wk_t = load_w("wk", wk, DH)
        wv_t = load_w("wv", wv, DH)
        wo_t = load_w("wo", wo, D)

        # --- persistent intermediates ------------------------------------
        # Q^T / K^T per head pair g: [128 (= 2 heads x 64), S]
        qts = [qkp.tile([128, S], BF16, name=f"qt{g}", tag=f"qt{g}") for g in range(4)]
        kts = [qkp.tile([128, S], BF16, name=f"kt{g}", tag=f"kt{g}") for g in range(4)]
        # V with interleaved ones column: [128 keys, 8 heads * 65]
        vts = [vp.tile([128, 8 * 65], BF16, name=f"v{m}", tag=f"v{m}") for m in range(NKB)]
        # normalized ctx^T per pair g: rows 0:64 head 2g, 64:128 head 2g+1
        cts = [ctp.tile([128, S], BF16, name=f"ct{g}", tag=f"ct{g}") for g in range(4)]

        def proj_qk_chunk(dst, w_tiles, g, c):
            """dst[:, CH*c:CH*(c+1)] = (W pair cols)^T X^T chunk, bf16."""
            ps = ps_mm.tile([128, CH], F32, name="mm", tag="mm")
            for k in range(KT):
                nc.tensor.matmul(
                    ps[:],
                    lhsT=w_tiles[k][:, 128 * g:128 * (g + 1)],
                    rhs=xts[k][:, CH * c:CH * (c + 1)],
                    start=(k == 0), stop=(k == KT - 1),
                )
            nc.vector.tensor_copy(dst[:, CH * c:CH * (c + 1)], ps[:])

        def proj_v_block(m):
            """V rows [128m, 128m+128) for all 8 heads, strided into vts[m]."""
            ps = ps_mm.tile([128, CH], F32, name="mm", tag="mm")
            for k in range(KT):
                nc.tensor.matmul(
                    ps[:],
                    lhsT=xts[k][:, 128 * m:128 * (m + 1)],
                    rhs=wv_t[k][:],
                    start=(k == 0), stop=(k == KT - 1),
                )
            vm = vts[m]
            dst = vm[:].rearrange("p (h x) -> p h x", x=65)[:, :, 0:64]
            src = ps[:].rearrange("p (h d) -> p h d", d=64)
            nc.vector.tensor_copy(dst, src)
            ones = vm[:].rearrange("p (h x) -> p h x", x=65)[:, :, 64:65]
            nc.vector.memset(ones, 1.0)

        def attention_chunk(g, c):
            """Heads (2g, 2g+1), queries [CH*c, CH*(c+1))."""
            qt, ktile = qts[g], kts[g]
            nj = 4 * c + 4
            ctx_e = ps_ctx.tile([65, CH], F32, name="ctx", tag="ctx")
            ctx_o = ps_ctx.tile([65, CH], F32, name="ctx", tag="ctx")
            for j in range(nj):
                d = j - 4 * c
                st = 128 * max(0, d)  # first valid q column in this chunk
                sc_e = ps_sc.tile([128, CH], F32, name="sc", tag="sc")
                sc_o = ps_sc.tile([128, CH], F32, name="sc", tag="sc")
                # scores^T = K_j Q^T  (row-packed pair: even rows 0:64, odd 64:128)
                nc.tensor.matmul(
                    sc_e[:, st:], lhsT=ktile[0:64, 128 * j:128 * (j + 1)],
                    rhs=qt[0:64, CH * c + st:CH * (c + 1)])
                nc.tensor.matmul(
                    sc_o[:, st:], lhsT=ktile[64:128, 128 * j:128 * (j + 1)],
                    rhs=qt[64:128, CH * c + st:CH * (c + 1)])
                if d >= 0:  # diagonal-crossing block: triangular mask
                    ms = slice(128 * d, 128 * (d + 1))
                    nc.vector.tensor_tensor(sc_e[:, ms], sc_e[:, ms], mask[:],
                                            op=mybir.AluOpType.add)
                    nc.vector.tensor_tensor(sc_o[:, ms], sc_o[:, ms], mask[:],
                                            op=mybir.AluOpType.add)
                pt_e = pTp.tile([128, CH], BF16, name="pT", tag="pT")
                pt_o = pTp.tile([128, CH], BF16, name="pT", tag="pT")
                nc.scalar.activation(pt_e[:, st:], sc_e[:, st:],
                                     mybir.ActivationFunctionType.Exp, scale=0.125)
                nc.scalar.activation(pt_o[:, st:], sc_o[:, st:],
                                     mybir.ActivationFunctionType.Exp, scale=0.125)
                he, ho = 2 * g, 2 * g + 1
                nc.tensor.matmul(ctx_e[:, st:], lhsT=vts[j][:, 65 * he:65 * he + 65],
                                 rhs=pt_e[:, st:], start=(j == 0), stop=(j == nj - 1),
                                 skip_group_check=True)
                nc.tensor.matmul(ctx_o[:, st:], lhsT=vts[j][:, 65 * ho:65 * ho + 65],
                                 rhs=pt_o[:, st:], start=(j == 0), stop=(j == nj - 1),
                                 skip_group_check=True)
            # normalize by the fused denominator row and store into ctxT (bf16)
            for par, (ctx_ps, rows) in enumerate(
                    ((ctx_e, slice(0, 64)), (ctx_o, slice(64, 128)))):
                idx = (g * NCH + c) * 2 + par
                # custom-DVE ops read garbage from PSUM on HW: stage via SBUF
                srow = rp.tile([1, CH], F32, name="srow", tag="srow")
                nc.vector.tensor_copy(srow[:], ctx_ps[64:65, :])
                rc = rp.tile([1, CH], F32, name="recip", tag="recip")
                nc.vector.reciprocal_approx_fast(rc[:], srow[:])
                nc.sync.dma_start(rscratch[idx:idx + 1, :], rc[:])
                bc = bp.tile([64, CH], F32, name="bcast", tag="bcast")
                nc.sync.dma_start(bc[:], rscratch[idx:idx + 1, :].broadcast_to([64, CH]))
                nc.vector.tensor_tensor(cts[g][rows, CH * c:CH * (c + 1)],
                                        ctx_ps[0:64, :], bc[:],
                                        op=mybir.AluOpType.mult)

        def outproj_chunk(c):
            for qb in range(4 * c, 4 * c + 4):
                for n in range(2):
                    ps = ps_mm.tile([128, CH], F32, name="mm", tag="mm")
                    for g in range(4):
                        nc.tensor.matmul(
                            ps[:], lhsT=cts[g][:, 128 * qb:128 * (qb + 1)],
                            rhs=wo_t[g][:, CH * n:CH * (n + 1)],
                            start=(g == 0), stop=(g == 3),
                        )
                    ot = osb.tile([128, CH], F32, name="osb", tag="osb")
                    nc.vector.tensor_copy(ot[:], ps[:])
                    nc.sync.dma_start(out[128 * qb:128 * (qb + 1), CH * n:CH * (n + 1)],
                                      ot[:])

        # --- schedule ----------------------------------------------------
        for m in range(NKB):
            proj_v_block(m)
        for c in range(NCH):
            proj_qk_chunk(kts[0], wk_t, 0, c)
            proj_qk_chunk(qts[0], wq_t, 0, c)
        for g in range(4):
            for c in range(NCH):
                attention_chunk(g, c)
                if g < 3:  # produce next pair's Q^T/K^T while ACT chews exps
                    proj_qk_chunk(kts[g + 1], wk_t, g + 1, c)
                    proj_qk_chunk(qts[g + 1], wq_t, g + 1, c)
                else:
                    outproj_chunk(c)

    nc.compile()
    return nc


def _get_module():
    global _CACHED
    if _CACHED is None:
        _CACHED = build_module()
    return _CACHED


def _causal_mask_tile():
    k = np.arange(128)[:, None]
    q = np.arange(128)[None, :]
    return np.where(k <= q, 0.0, NEG).astype(np.float32)


def kernel(inputs, Wq, Wk, Wv, Wo, bo):
    inputs = np.asarray(inputs, dtype=np.float32)
    Wq = np.asarray(Wq, dtype=np.float32)
    Wk = np.asarray(Wk, dtype=np.float32)
    Wv = np.asarray(Wv, dtype=np.float32)
    Wo = np.asarray(Wo, dtype=np.float32)
    bo = np.asarray(bo, dtype=np.float32)

    bf = ml_dtypes.bfloat16
    mask = _causal_mask_tile()
    in_maps = []
    for c in range(NCORES):
        b, hh = c // 2, c % 2
        cols = slice(DH * hh, DH * (hh + 1))
        in_maps.append({
            "xT": np.ascontiguousarray(inputs[b].T).astype(bf),
            "wq": np.ascontiguousarray(Wq[:, cols]).astype(bf),
            "wk": np.ascontiguousarray(Wk[:, cols]).astype(bf),
            "wv": np.ascontiguousarray(Wv[:, cols]).astype(bf),
            "wo": np.ascontiguousarray(Wo[cols, :]).astype(bf),
            "maskt": mask,
        })

    nc = _get_module()
    res = bass_utils.run_bass_kernel_spmd(nc, in_maps, core_ids=list(range(NCORES)))
    outs = [r["out"] for r in res.results]

    full = np.empty((B, S, D), dtype=np.float32)
    for b in range(B):
        full[b] = outs[2 * b] + outs[2 * b + 1] + bo[None, :]
    return full
